# revision 1
# baseline (speedup 1.0000x reference)
"""Expert-parallel MoE SwiGLU kernel for Trainium2 (8 NeuronCores).

Strategy: each of the 8 cores owns one expert's weights (w1/w3/w2).  Token
routing (the "all-to-all dispatch") is done host-side: tokens are gathered
per expert, padded to a common capacity T, and each core computes

    y_e = (silu(x_e @ w1_e) * (x_e @ w3_e)) @ w2_e          # [T, H]

for its expert's token set.  The host then scatter-adds the weighted
per-expert outputs back into the [B, H] result.  Matmuls run in float32r
(full-rate fp32 mode on the PE array); all data stays fp32 end to end.
"""

import numpy as np

_P = 128
_E = 8  # experts == cores

# (H, I, T) -> compiled Bass program
_PROG_CACHE = {}
# test hooks: set TRACE=True before calling kernel() to capture an NTFF
# profile; the BassKernelResults of the last run lands in LAST_RUN.
TRACE = False
LAST_RUN = None


def _build_program(H, I, T):
    import concourse.bass as bass
    import concourse.tile as tile
    from concourse import bacc, mybir

    f32 = mybir.dt.float32
    f32r = mybir.dt.float32r
    Sigmoid = mybir.ActivationFunctionType.Sigmoid
    ts = bass.ts

    HC = H // _P
    IC = I // _P
    assert H % _P == 0 and I % _P == 0 and T % 16 == 0

    # token free-dim tiling (both phases): chunks of <=512, as equal as
    # possible (so chunks stay >=256 and f32r matmuls keep 1 cycle/row)
    nt = -(-T // 512)
    q, r = divmod(T, nt)
    fsz = [q + (1 if i < r else 0) for i in range(nt)]
    foff = [sum(fsz[:i]) for i in range(nt)]

    nc = bacc.Bacc(
        "TRN2",
        target_bir_lowering=False,
        debug=False,
        enable_asserts=False,
        num_devices=_E,
    )
    # inputs are declared float32r (same bits as fp32 on the numpy side) so
    # the BIR verifier sees a consistent f32r producer chain into the
    # full-rate f32r matmuls
    xT = nc.dram_tensor("xT", [H, T], f32r, kind="ExternalInput").ap()
    w1 = nc.dram_tensor("w1", [H, I], f32r, kind="ExternalInput").ap()
    w3 = nc.dram_tensor("w3", [H, I], f32r, kind="ExternalInput").ap()
    w2 = nc.dram_tensor("w2", [I, H], f32r, kind="ExternalInput").ap()
    # output is y^T [H, T]: phase 2 accumulates with H on partitions so the
    # token dim needs no 128-granularity (capacity T can hug max_count)
    y = nc.dram_tensor("y", [H, T], f32, kind="ExternalOutput").ap()

    # half-block weight tiles: w_bufs//2 i-blocks of DMA lookahead
    w_bufs = 6 if T <= 544 else 4
    w2_bufs = 3

    with tile.TileContext(nc) as tc:
        with (
            tc.tile_pool(name="xp", bufs=1) as xp,
            tc.tile_pool(name="cp", bufs=1) as cp,
            tc.tile_pool(name="wp", bufs=w_bufs) as wp,
            tc.tile_pool(name="w2p", bufs=w2_bufs) as w2p,
            tc.tile_pool(name="hp", bufs=1) as hp,
            tc.tile_pool(name="sp", bufs=2) as sp,
            tc.tile_pool(name="op", bufs=4) as op,
            tc.tile_pool(name="pp", bufs=8, space="PSUM") as pp,
        ):
            zbias = cp.tile([_P, 1], f32)
            nc.any.memset(zbias[:], 0.0)

            # resident activations: x^T as [p, hc, t], h^T as [p, ic, t].
            # x loads as 4 chunks spread over different engine queues so the
            # first chunks land fast and the first accumulation group can
            # start without waiting for the whole 4MB.
            xTr = xT.rearrange("(hc p) t -> p hc t", p=_P)
            n_xc = 2 if HC % 2 == 0 else 1
            xcs = HC // n_xc
            x_engs = [nc.sync, nc.scalar]
            xs_chunks = []
            for c in range(n_xc):
                xc = xp.tile([_P, xcs, T], f32r, tag=f"xs{c}", name=f"xs_{c}")
                x_engs[c % 2].dma_start(xc[:], xTr[:, c * xcs : (c + 1) * xcs, :])
                xs_chunks.append(xc)

            def xs_slice(hc, lo, hi):
                return xs_chunks[hc // xcs][:, hc % xcs, lo:hi]

            hs = hp.tile([_P, IC, T], f32r)

            w1r = w1.rearrange("(hc p) i -> p hc i", p=_P)
            w3r = w3.rearrange("(hc p) i -> p hc i", p=_P)

            # ---- phase 1: h^T[i, t] = silu(w1^T x)[i, t] * (w3^T x)[i, t]
            # w1/w3 stream per 128-wide i-block in quarter-blocks so the PE
            # can start on the first 0.5MB and the DMA pipeline stays fine-
            # grained (each quarter is its own pool slot / dependency)
            WQ = 2 if HC % 2 == 0 else 1
            HCQ = HC // WQ
            for ic in range(IC):
                w1q = []
                w3q = []
                for qq in range(WQ):
                    w1s = wp.tile([_P, HCQ, _P], f32r, tag="w1", name=f"w1s_{ic}_{qq}")
                    nc.sync.dma_start(
                        w1s[:], w1r[:, qq * HCQ : (qq + 1) * HCQ, ts(ic, _P)]
                    )
                    w1q.append(w1s)
                    w3s = wp.tile([_P, HCQ, _P], f32r, tag="w3", name=f"w3s_{ic}_{qq}")
                    nc.scalar.dma_start(
                        w3s[:], w3r[:, qq * HCQ : (qq + 1) * HCQ, ts(ic, _P)]
                    )
                    w3q.append(w3s)
                for ti, (off, ft) in enumerate(zip(foff, fsz)):
                    pg = pp.tile([_P, 512], f32, tag="ps", name=f"pg_{ic}_{ti}")
                    pu = pp.tile([_P, 512], f32, tag="ps", name=f"pu_{ic}_{ti}")
                    for hc in range(HC):
                        nc.tensor.matmul(
                            pg[:, :ft],
                            lhsT=w1q[hc // HCQ][:, hc % HCQ, :],
                            rhs=xs_slice(hc, off, off + ft),
                            start=(hc == 0),
                            stop=(hc == HC - 1),
                        )
                    for hc in range(HC):
                        nc.tensor.matmul(
                            pu[:, :ft],
                            lhsT=w3q[hc // HCQ][:, hc % HCQ, :],
                            rhs=xs_slice(hc, off, off + ft),
                            start=(hc == 0),
                            stop=(hc == HC - 1),
                        )
                    # silu(g) * u  ==  sigmoid(g) * g * u
                    sig = sp.tile([_P, 512], f32, tag="sig", name=f"sig_{ic}_{ti}")
                    nc.scalar.activation(sig[:, :ft], pg[:, :ft], Sigmoid, bias=zbias[:])
                    gs = sp.tile([_P, 512], f32, tag="gs", name=f"gs_{ic}_{ti}")
                    nc.vector.tensor_mul(gs[:, :ft], sig[:, :ft], pg[:, :ft])
                    nc.vector.tensor_mul(
                        hs[:, ic, off : off + ft], gs[:, :ft], pu[:, :ft]
                    )

            # ---- phase 2: y^T[h, t] = sum_i w2[i, h] * h^T[i, t]
            # stationary = w2 sub-blocks [128 (i), 128 (h)], moving = h^T
            # slices; accumulate over i in PSUM with h on partitions.
            w2r = w2.rearrange("(ic p) h -> p ic h", p=_P)
            ICH = IC // 2  # stream w2 per output h-chunk in two half-blocks
            for hc2 in range(HC):
                pys = [
                    pp.tile([_P, 512], f32, tag="ps", name=f"py_{hc2}_{ti}")
                    for ti in range(nt)
                ]
                for half in range(2):
                    w2s = w2p.tile(
                        [_P, ICH, _P], f32r, tag="w2", name=f"w2s_{hc2}_{half}"
                    )
                    # alternate between the two HWDGE rings
                    dma_eng = nc.sync if (2 * hc2 + half) % 2 == 0 else nc.scalar
                    dma_eng.dma_start(
                        w2s[:], w2r[:, half * ICH : (half + 1) * ICH, ts(hc2, _P)]
                    )
                    for ich in range(ICH):
                        ic = half * ICH + ich
                        for ti, (off, ft) in enumerate(zip(foff, fsz)):
                            nc.tensor.matmul(
                                pys[ti][:, :ft],
                                lhsT=w2s[:, ich, :],
                                rhs=hs[:, ic, off : off + ft],
                                start=(ic == 0),
                                stop=(ic == IC - 1),
                            )
                for ti, (off, ft) in enumerate(zip(foff, fsz)):
                    ot = op.tile([_P, 512], f32, tag="ot", name=f"ot_{hc2}_{ti}")
                    nc.vector.tensor_copy(ot[:, :ft], pys[ti][:, :ft])
                    nc.scalar.dma_start(y[ts(hc2, _P), off : off + ft], ot[:, :ft])

    nc.compile()
    return nc


def _get_program(H, I, T):
    key = (H, I, T)
    if key not in _PROG_CACHE:
        _PROG_CACHE[key] = _build_program(H, I, T)
    return _PROG_CACHE[key]


def kernel(x, expert_indices, expert_weights, w1, w2, w3):
    global LAST_RUN
    from concourse.bass_utils import run_bass_kernel_spmd

    x = np.ascontiguousarray(np.asarray(x, dtype=np.float32))
    idx = np.asarray(expert_indices)
    idx_dtype = idx.dtype
    idx = idx.astype(np.int64)
    wts = np.asarray(expert_weights, dtype=np.float32)
    w1 = np.asarray(w1, dtype=np.float32)
    w2 = np.asarray(w2, dtype=np.float32)
    w3 = np.asarray(w3, dtype=np.float32)

    B, H = x.shape
    E, _, I = w1.shape
    assert E == _E, f"expected {_E} experts, got {E}"
    K = idx.shape[1]

    # host-side dispatch: per-token expert weight matrix (merges duplicate
    # top-k hits of the same expert), then token lists per expert
    wmat = np.zeros((B, E), np.float32)
    np.add.at(wmat, (np.arange(B)[:, None], idx), wts)
    sel = np.zeros((B, E), bool)
    sel[np.arange(B)[:, None], idx] = True

    toks = [np.nonzero(sel[:, e])[0] for e in range(E)]
    max_count = max(len(t) for t in toks)

    # capacity per round: SBUF residency (x^T and h^T tiles) caps T
    cap_limit = 608
    rounds = max(1, -(-max_count // cap_limit))
    per_round = -(-max_count // rounds)
    T = max(256, -(-per_round // 16) * 16)

    nc = _get_program(H, I, T)
    xTfull = np.ascontiguousarray(x.T)  # [H, B]

    out = np.zeros((B, H), np.float32)
    for rd in range(rounds):
        in_maps = []
        rtoks = []
        for e in range(E):
            te = toks[e][rd * per_round : (rd + 1) * per_round]
            rtoks.append(te)
            xTe = np.zeros((H, T), np.float32)
            if len(te):
                xTe[:, : len(te)] = xTfull[:, te]
            in_maps.append(
                {
                    "xT": xTe,
                    "w1": np.ascontiguousarray(w1[e]),
                    "w3": np.ascontiguousarray(w3[e]),
                    "w2": np.ascontiguousarray(w2[e]),
                }
            )
        res = run_bass_kernel_spmd(nc, in_maps, list(range(_E)), trace=TRACE)
        LAST_RUN = res
        for e in range(E):
            te = rtoks[e]
            if len(te):
                ye = res.results[e]["y"][:, : len(te)].T  # y^T [H, T] -> [n, H]
                out[te] += wmat[te, e][:, None] * ye

    return out



# revision 3
# speedup vs baseline: 1.0476x; 1.0476x over previous
"""Expert-sliced MoE SwiGLU kernel for Trainium2 (8 NeuronCores), v2.

Strategy (I-slice data layout): every core owns a 512-wide slice of the
intermediate dimension of ALL 8 experts.  Core c computes, for every
expert e and its routed tokens x_e:

    h_e[c]  = silu(x_e @ w1_e[:, sl_c]) * (x_e @ w3_e[:, sl_c])   # [T_e, 512]
    y_e[c]  = h_e[c] @ w2_e[sl_c, :]                              # [T_e, H] partial

The host sums the 8 partial y's and scatter-adds into [B, H].  This makes
every core execute the *same* total work (sum over experts), eliminating
the max-expert-count padding of pure expert-parallel.  All matmul operands
are bf16 (fp32 PSUM accumulation), which halves weight DMA vs fp32 and
enables fast-weight-load; per-matmul rate is identical (1 col/cycle).

DMA queues drain FIFO at ~225 GB/s each (~450 GB/s chip): weights stream
on the sync queue (w1) and vector queue (w3), x + w2 on scalar (x first —
it gates compute), y out on gpsimd.  The first expert's tiles are split
into pieces so the first matmul group starts as soon as its first piece
lands, and dummy matmuls on a zero tile keep the PE busy through the
initial DMA so the HAM clock-gate opens before real work begins.
"""

import numpy as np

_P = 128
_E = 8            # experts
_H = 2048         # hidden
_I = 4096         # intermediate (full)
_ISL = _I // 8    # per-core i-slice
_HC = _H // _P    # 16 h-blocks
_IC = _ISL // _P  # 4 i-blocks per slice
_CMAX = 512       # max tokens per chunk (one PSUM bank of fp32)

_PROG_CACHE = {}
TRACE = False
LAST_RUN = None

try:
    import ml_dtypes

    _BF16 = np.dtype(ml_dtypes.bfloat16)
except Exception:  # pragma: no cover
    _BF16 = None


def _f32_to_bf16(a):
    """Round-to-nearest-even fp32 -> bf16 (as ml_dtypes.bfloat16 view)."""
    a = np.ascontiguousarray(a, dtype=np.float32)
    u = a.view(np.uint32)
    r = (u >> 16) & np.uint32(1)
    b = ((u + np.uint32(0x7FFF) + r) >> 16).astype(np.uint16)
    return b.view(_BF16)


def _bf16_to_f32(b):
    u = np.asarray(b).view(np.uint16).astype(np.uint32) << 16
    return u.view(np.float32)


def _chunk_sizes(T):
    """Split padded count T (multiple of 16) into chunks <= _CMAX, equal-ish,
    each a multiple of 16."""
    if T == 0:
        return []
    nt = -(-T // _CMAX)
    q, r = divmod(T // 16, nt)
    return [(q + (1 if i < r else 0)) * 16 for i in range(nt)]


def _build_program(jobs, XW):
    """jobs: tuple of (expert, ct, xcol, new_expert); XW = total packed width
    of the x/y dram buffers (sum of 16*ct)."""
    import concourse.tile as tile
    from concourse import bacc, mybir

    f32 = mybir.dt.float32
    bf16 = mybir.dt.bfloat16
    Silu = mybir.ActivationFunctionType.Silu
    Copy = mybir.ActivationFunctionType.Copy

    WW = _IC * _HC * _P  # per-expert packed width of w1/w3 (= 8192), ic-major
    W2W = _IC * _H       # per-expert packed width of w2 (= 8192)

    nc = bacc.Bacc(
        "TRN2",
        target_bir_lowering=False,
        debug=False,
        enable_asserts=False,
        num_devices=_E,
    )
    # All dram tensors are laid out [128, n] with fully-contiguous
    # per-partition rows (host pre-transposes), so every DMA is a maximal
    # 2D burst.
    xd = nc.dram_tensor("xd", [_P, XW], bf16, kind="ExternalInput").ap()
    w1d = nc.dram_tensor("w1d", [_P, _E * WW], bf16, kind="ExternalInput").ap()
    w3d = nc.dram_tensor("w3d", [_P, _E * WW], bf16, kind="ExternalInput").ap()
    w2d = nc.dram_tensor("w2d", [_P, _E * W2W], bf16, kind="ExternalInput").ap()
    yd = nc.dram_tensor("yd", [_P, XW], bf16, kind="ExternalOutput").ap()

    with tile.TileContext(nc) as tc:
        with (
            tc.tile_pool(name="cp", bufs=1) as cp,
            tc.tile_pool(name="xp", bufs=3) as xp,
            tc.tile_pool(name="wp", bufs=2) as wp,
            tc.tile_pool(name="hp", bufs=2) as hp,
            tc.tile_pool(name="sp", bufs=2) as sp,
            tc.tile_pool(name="op", bufs=2) as op,
            tc.tile_pool(name="pp", bufs=1, space="PSUM") as pp,
        ):
            # ---- HAM prewarm: dummy matmuls on a zero tile keep the PE
            # busy during the initial weight/x DMA so the clock gate opens
            # (K=8/8) before the first real matmul.  They share the "py"
            # PSUM banks (all done long before phase 2 starts).
            wz = cp.tile([_P, _P], bf16)
            nc.vector.memset(wz[:], 0.0)
            for k in range(30):
                pw = pp.tile([_P, _CMAX], f32, tag="py", bufs=4, name=f"pw_{k}")
                nc.tensor.matmul(
                    pw[:, : _P], lhsT=wz[:], rhs=wz[:], start=True, stop=True
                )

            njobs = len(jobs)
            xts, w1ts, w3ts, w2ts, hts = {}, {}, {}, {}, {}
            wqs = {}  # expert -> weight queue (alternates sync/scalar)

            def load_job(j):
                e, ct, xcol, new_e = jobs[j]
                if new_e:
                    # alternate each expert's weights between the two hardware
                    # queues so weight bandwidth rides both DGE rings; x rides
                    # the opposite queue from its expert's weights.
                    wq = nc.sync if len(wqs) % 2 == 0 else nc.scalar
                    wqs[e] = wq
                xq = nc.scalar if wqs[e] is nc.sync else nc.sync
                xt = xp.tile([_P, _HC * _CMAX], bf16, tag="x", name=f"xt_{j}")
                if j == 0:
                    # first x in hc-quarters split across both queues so the
                    # first accumulation group starts as soon as possible
                    # (compute consumes hc-major)
                    Q = _HC // 4
                    for q in range(4):
                        eng = wqs[e] if q < 2 else xq
                        eng.dma_start(
                            xt[:, q * Q * ct : (q + 1) * Q * ct],
                            xd[:, xcol + q * Q * ct : xcol + (q + 1) * Q * ct],
                        )
                else:
                    xq.dma_start(xt[:, : _HC * ct], xd[:, xcol : xcol + _HC * ct])
                xts[j] = xt
                if new_e:
                    # w1+w3 interleaved as per-ic pieces (contiguous: packing
                    # is ic-major) in exactly the order phase 1 consumes them,
                    # so the first accumulation group starts on piece 0.
                    wq = wqs[e]
                    w1t = wp.tile([_P, WW], bf16, tag="w1", name=f"w1t_{e}")
                    w3t = wp.tile([_P, WW], bf16, tag="w3", name=f"w3t_{e}")
                    PW = WW // _IC
                    for ic in range(_IC):
                        wq.dma_start(
                            w1t[:, ic * PW : (ic + 1) * PW],
                            w1d[:, e * WW + ic * PW : e * WW + (ic + 1) * PW],
                        )
                        wq.dma_start(
                            w3t[:, ic * PW : (ic + 1) * PW],
                            w3d[:, e * WW + ic * PW : e * WW + (ic + 1) * PW],
                        )
                    w1ts[e], w3ts[e] = w1t, w3t

            def load_w2(j):
                # deferred one job so it never delays the next x / w13 loads
                e, _, _, new_e = jobs[j]
                if new_e:
                    w2t = wp.tile([_P, W2W], bf16, tag="w2", name=f"w2t_{e}")
                    wqs[e].dma_start(w2t[:], w2d[:, e * W2W : (e + 1) * W2W])
                    w2ts[e] = w2t

            def ph1(j):
                e, ct, xcol, _ = jobs[j]
                xt, w1t, w3t = xts[j], w1ts[e], w3ts[e]
                ht = hp.tile([_P, _IC * _CMAX], bf16, tag="h", name=f"ht_{j}")
                hts[j] = ht
                for ic in range(_IC):
                    pg = pp.tile([_P, _CMAX], f32, tag="pg", bufs=2, name=f"pg_{j}_{ic}")
                    pu = pp.tile([_P, _CMAX], f32, tag="pu", bufs=2, name=f"pu_{j}_{ic}")
                    for hc in range(_HC):
                        nc.tensor.matmul(
                            pg[:, :ct],
                            lhsT=w1t[:, (ic * _HC + hc) * _P : (ic * _HC + hc + 1) * _P],
                            rhs=xt[:, hc * ct : (hc + 1) * ct],
                            start=(hc == 0),
                            stop=(hc == _HC - 1),
                        )
                    for hc in range(_HC):
                        nc.tensor.matmul(
                            pu[:, :ct],
                            lhsT=w3t[:, (ic * _HC + hc) * _P : (ic * _HC + hc + 1) * _P],
                            rhs=xt[:, hc * ct : (hc + 1) * ct],
                            start=(hc == 0),
                            stop=(hc == _HC - 1),
                        )
                    sg = sp.tile([_P, _CMAX], f32, tag="sg", name=f"sg_{j}_{ic}")
                    nc.scalar.activation(sg[:, :ct], pg[:, :ct], Silu)
                    nc.vector.tensor_mul(
                        ht[:, ic * ct : (ic + 1) * ct], sg[:, :ct], pu[:, :ct]
                    )

            def ph2(j, last=False):
                e, ct, xcol, _ = jobs[j]
                ht, w2t = hts[j], w2ts[e]
                ot = op.tile([_P, _HC * _CMAX], bf16, tag="ot", name=f"ot_{j}")
                for hc in range(_HC):
                    py = pp.tile([_P, _CMAX], f32, tag="py", bufs=4, name=f"py_{j}_{hc}")
                    for ic in range(_IC):
                        nc.tensor.matmul(
                            py[:, :ct],
                            lhsT=w2t[:, ic * _H + hc * _P : ic * _H + (hc + 1) * _P],
                            rhs=ht[:, ic * ct : (ic + 1) * ct],
                            start=(ic == 0),
                            stop=(ic == _IC - 1),
                        )
                    # alternate the PSUM->SBUF cast between scalar and vector
                    if hc % 2 == 0:
                        nc.scalar.activation(
                            ot[:, hc * ct : (hc + 1) * ct], py[:, :ct], Copy
                        )
                    else:
                        nc.vector.tensor_copy(
                            ot[:, hc * ct : (hc + 1) * ct], py[:, :ct]
                        )
                    # stream output quarters on the gpsimd (software) queue;
                    # the last job flushes eighths on the scalar hardware
                    # queue (idle by then) so the final drain is short.
                    if last:
                        if hc % 2 == 1:
                            nc.scalar.dma_start(
                                yd[:, xcol + (hc - 1) * ct : xcol + (hc + 1) * ct],
                                ot[:, (hc - 1) * ct : (hc + 1) * ct],
                            )
                    elif hc % 4 == 3:
                        nc.gpsimd.dma_start(
                            yd[:, xcol + (hc - 3) * ct : xcol + (hc + 1) * ct],
                            ot[:, (hc - 3) * ct : (hc + 1) * ct],
                        )

            # software pipeline: ph1 runs one job ahead of ph2 so the PE
            # never waits on the silu/mul chain of the current job.
            # software pipeline, x/weights preloaded two jobs ahead
            load_job(0)
            if njobs > 1:
                load_job(1)
            load_w2(0)
            ph1(0)
            for j in range(1, njobs):
                if j + 1 < njobs:
                    load_job(j + 1)
                load_w2(j)
                ph1(j)
                ph2(j - 1)
            ph2(njobs - 1, last=True)

    nc.compile()
    return nc


def _get_program(jobs, XW):
    key = (jobs, XW)
    if key not in _PROG_CACHE:
        _PROG_CACHE[key] = _build_program(jobs, XW)
    return _PROG_CACHE[key]


def kernel(x, expert_indices, expert_weights, w1, w2, w3):
    global LAST_RUN
    from concourse.bass_utils import run_bass_kernel_spmd

    x = np.ascontiguousarray(np.asarray(x, dtype=np.float32))
    idx = np.asarray(expert_indices).astype(np.int64)
    wts = np.asarray(expert_weights, dtype=np.float32)
    w1 = np.asarray(w1, dtype=np.float32)
    w2 = np.asarray(w2, dtype=np.float32)
    w3 = np.asarray(w3, dtype=np.float32)

    B, H = x.shape
    E, _, I = w1.shape
    assert (E, H, I) == (_E, _H, _I), f"unexpected shapes {(E, H, I)}"

    # ---- host-side dispatch: merge duplicate top-k hits, gather per expert
    wmat = np.zeros((B, E), np.float32)
    np.add.at(wmat, (np.arange(B)[:, None], idx), wts)
    sel = np.zeros((B, E), bool)
    sel[np.arange(B)[:, None], idx] = True
    toks = [np.nonzero(sel[:, e])[0] for e in range(E)]

    # ---- job list: (expert, chunk_tokens, x/y column offset, new_expert)
    jobs = []
    host_jobs = []  # (e, ct, xcol, tok_start)
    xcol = 0
    # biggest experts first: a multi-chunk expert leads, giving the DMA
    # ramp two PE jobs on one weight load; the smallest expert's short
    # final job shrinks the tail drain.
    order = sorted(range(E), key=lambda e: -len(toks[e]))
    for e in order:
        T = len(toks[e])
        Tp = -(-T // 16) * 16
        start = 0
        for ci, ct in enumerate(_chunk_sizes(Tp)):
            jobs.append((e, ct, xcol, ci == 0))
            host_jobs.append((e, ct, xcol, start))
            xcol += _HC * ct
            start += ct
    XW = xcol
    assert jobs, "no routed tokens"

    # ---- pack x: bf16, gathered by (expert, chunk), laid out per chunk as
    # [128 partitions, hc, t] flattened -> fully contiguous DMA rows
    xb = _f32_to_bf16(x)  # [B, H] bf16
    xd = np.zeros((_P, XW), dtype=_BF16)
    for (e, ct, col, tstart) in host_jobs:
        te = toks[e][tstart : tstart + ct]
        blk = np.zeros((ct, _H), dtype=_BF16)
        blk[: len(te)] = xb[te]
        # [t, hc*P] -> [P, hc, t]
        xd[:, col : col + _HC * ct] = (
            blk.reshape(ct, _HC, _P).transpose(2, 1, 0).reshape(_P, _HC * ct)
        )

    # ---- per-core weight slices, pre-transposed to [128, packed] layout
    w1b = _f32_to_bf16(w1)  # [E, H, I]
    w3b = _f32_to_bf16(w3)
    w2b = _f32_to_bf16(w2)  # [E, I, H]
    WW = _IC * _HC * _P
    W2W = _IC * _H
    in_maps = []
    for c in range(_E):
        sl = slice(c * _ISL, (c + 1) * _ISL)
        # w1/w3: [E, H, ISL] -> [P, E, IC, HC, P] (ic-major pieces):
        # w[p, e, ic, hc, i] = w1[e, hc*128+p, c*512 + ic*128 + i]
        w1c = (
            w1b[:, :, sl]
            .reshape(_E, _HC, _P, _IC, _P)
            .transpose(2, 0, 3, 1, 4)
        ).reshape(_P, _E * WW)
        w3c = (
            w3b[:, :, sl]
            .reshape(_E, _HC, _P, _IC, _P)
            .transpose(2, 0, 3, 1, 4)
        ).reshape(_P, _E * WW)
        # w2: [E, ISL, H] -> [P, E, IC, H]: w[p, e, ic, h] = w2[e, ic*128+p, h]
        w2c = (
            w2b[:, sl, :].reshape(_E, _IC, _P, _H).transpose(2, 0, 1, 3)
        ).reshape(_P, _E * W2W)
        in_maps.append(
            {
                "xd": xd,
                "w1d": np.ascontiguousarray(w1c),
                "w3d": np.ascontiguousarray(w3c),
                "w2d": np.ascontiguousarray(w2c),
            }
        )

    nc = _get_program(tuple(jobs), XW)
    res = run_bass_kernel_spmd(nc, in_maps, list(range(_E)), trace=TRACE)
    LAST_RUN = res

    # ---- reduce partial y over cores, decode, scatter-add
    ysum = np.zeros((_P, XW), np.float32)
    for c in range(_E):
        ysum += _bf16_to_f32(res.results[c]["yd"])

    out = np.zeros((B, H), np.float32)
    for (e, ct, col, tstart) in host_jobs:
        te = toks[e][tstart : tstart + ct]
        n = len(te)
        if n == 0:
            continue
        # [P, hc, t] -> [t, hc*P] = [t, H]
        yblk = (
            ysum[:, col : col + _HC * ct]
            .reshape(_P, _HC, ct)
            .transpose(2, 1, 0)
            .reshape(ct, _H)
        )
        out[te] += wmat[te, e][:, None] * yblk[:n]

    return out
